# revision 43
# baseline (speedup 1.0000x reference)
"""PVT-style spatial-reduction attention on 8 Trainium2 NeuronCores.

Sharding: data-parallel over batch (B=8 -> one batch element per core).
Each core runs the full attention for its batch element; weights are
replicated. No collectives needed.

Token space is PERMUTED on device: the host ships x as conv patches
pat[s*C+c, p] = x^T[c, token(s, p)] (s = 2x2 tap index, p = patch
position), so the conv and the q projection read the same tensor and
no separate x^T upload/transpose is needed. All q-indexed tensors
(qT, scores, attnT, out) live in permuted token order; the host
un-permutes the output rows.

Per-core pipeline (matmuls f16, f32 PSUM accumulation):
  conv2x2s2 + LayerNorm -> ln (kv tokens), lnT via PE transpose
  kT = Wk^T lnT ; v = ln @ Wv (v_aug carries a ones column -> softmax
  denominators fall out of the attn@v matmul)
  scores S^T = kT.T qT in 512-col blocks -> two 3-bank PSUM regions;
  exp([128,1536]) per region on the Scalar engine (the ~200us floor
  this kernel hides everything under)
  attn@v accumulated kvc-wise; 1/denom via DVE reciprocal + gpsimd
  partition broadcast; out = attnT.T @ Wp + bp, f16 DMA out.
"""

import os
import sys
import numpy as np

for _p in ("/opt/trn_rl_repo", "/root/.axon_site/_ro/trn_rl_repo"):
    if os.path.isdir(_p) and _p not in sys.path:
        sys.path.append(_p)

import concourse.bacc as bacc
import concourse.bass as bass
import concourse.mybir as mybir
import concourse.tile as tile
from concourse.bass_utils import run_bass_kernel_spmd
from concourse.masks import make_identity

F16 = mybir.dt.float16
F32 = mybir.dt.float32

N = 4096          # q tokens (H*W = 64*64)
C = 320           # model dim
NH = 5            # heads
HD = 64           # head dim
NP = 1024         # kv tokens ((H/2)*(W/2))
LN_EPS = 1e-3
SCALE = HD ** -0.5
EXP_BIAS = -3.0   # constant shift inside exp; cancels in softmax

# contraction chunks over C=320: three tiles; the last holds c 256:320
# in partitions 64:128 (so tile_position/base checks line up).
CCHUNKS = [(0, 0), (128, 0), (192, 64)]  # (c_start, row0); rows r0:128 used
OCHUNKS = [(0, 128), (128, 128), (256, 64)]

QBB = 2048        # max q block per attention unit
# attention q blocks (base, size): a small final block keeps the un-
# overlapped AV+proj tail short
QBS = [(0, 2048), (2048, 1536), (3584, 512)]


def _regs(nblocks):
    r = [3] * (nblocks // 3)
    if nblocks % 3:
        r.append(nblocks % 3)
    return r


def build_bass(dbg=False):
    nc = bacc.Bacc("TRN2", target_bir_lowering=False, debug=False, num_devices=8)

    pat_d = nc.declare_dram_parameter("pat", [4 * C, NP], F16, isOutput=False)
    wq_d = nc.declare_dram_parameter("wq", [C, C], F16, isOutput=False)
    wk_d = nc.declare_dram_parameter("wk", [C, C], F16, isOutput=False)
    wv_d = nc.declare_dram_parameter("wv", [C, C], F16, isOutput=False)
    srw_d = nc.declare_dram_parameter("srw", [4 * C, C], F16, isOutput=False)
    wp_d = nc.declare_dram_parameter("wp", [C, C], F16, isOutput=False)
    srb_d = nc.declare_dram_parameter("srb", [C], F16, isOutput=False)
    bk_d = nc.declare_dram_parameter("bk", [C], F32, isOutput=False)
    bv_d = nc.declare_dram_parameter("bv", [C], F16, isOutput=False)
    bp_d = nc.declare_dram_parameter("bp", [C], F16, isOutput=False)
    out_d = nc.declare_dram_parameter("out", [N, C], F16, isOutput=True)
    if dbg:
        dbg_d = {
            nm: nc.declare_dram_parameter(nm, shp, F16, isOutput=True)
            for nm, shp in (
                [(f"dbg_qt{i}", [128, N]) for i in range(3)]
                + [(f"dbg_kt{i}", [128, NP]) for i in range(3)]
                + [(f"dbg_at{i}", [128, QBB]) for i in range(3)]
            )
        }

    with tile.TileContext(nc) as tc:
        with (
            tc.tile_pool(name="consts", bufs=1) as consts,
            tc.tile_pool(name="wpool", bufs=1) as wpool,
            tc.tile_pool(name="big", bufs=1) as bigp,
            tc.tile_pool(name="sexp", bufs=24) as sexp_p,
            tc.tile_pool(name="attn", bufs=2) as attn_p,
            tc.tile_pool(name="small", bufs=4) as small_p,
            tc.tile_pool(name="normp", bufs=2) as norm_p,
            tc.tile_pool(name="outp", bufs=3) as out_p,
            # PSUM: two 3-bank score regions + one 2-bank rotating pool
            # for everything else = exactly 8 banks.
            tc.tile_pool(name="ps_sc", bufs=2, space="PSUM") as ps_sc,
            tc.tile_pool(name="ps_av", bufs=2, space="PSUM") as ps_av,
        ):
            scAB = [ps_sc.tile([128, 1536], F32, name=f"sc{i}", tag="sc")
                    for i in range(2)]

            # PE warmup: junk matmuls before any DMA lands, so the HAM
            # clock gate sees a busy PE and lifts to 2.4 GHz early.
            warm_sb = consts.tile([128, 512], F16, name="warm_sb")
            nc.vector.memset(warm_sb, 0.0)
            warm_ps = ps_av.tile([128, 512], F32, name="warm_ps", tag="a")
            for _w in range(12):
                nc.tensor.matmul(warm_ps, warm_sb[:, 0:128], warm_sb,
                                 start=True, stop=True)

            # preload the exp activation table (~2.7us) during the DMA wait
            ebias_t = consts.tile([128, 1], F32, name="ebias_t")
            nc.vector.memset(ebias_t, EXP_BIAS)
            warm_act = consts.tile([128, 2], F16, name="warm_act")
            nc.scalar.activation(warm_act, ebias_t.broadcast_to([128, 2]),
                                 mybir.ActivationFunctionType.Exp,
                                 bias=ebias_t, scale=1.0)

            # conv patches / permuted x^T, per (tap s, c-chunk cc), split
            # across both HWDGE queues. This is the only copy of x.
            pat_sb = []   # [s][cc] -> tile [128, NP] (cc2 rows at 64:128)
            for s in range(4):
                row = []
                for cc, (c0, r0) in enumerate(CCHUNKS):
                    t = bigp.tile([128, NP], F16, name=f"pat{s}_{cc}")
                    eng = nc.sync if (s * 3 + cc) % 2 else nc.scalar
                    eng.dma_start(
                        out=t[r0:128, :],
                        in_=pat_d[s * C + c0 + r0:s * C + c0 + 128, :])
                    row.append(t)
                pat_sb.append(row)

            # sr_w per (s, cc) with rows matching pat chunk partitions
            srw_sb = []
            for s in range(4):
                row = []
                for cc, (c0, r0) in enumerate(CCHUNKS):
                    t = wpool.tile([128, C], F16, name=f"srw{s}_{cc}")
                    eng = nc.scalar if (s * 3 + cc) % 2 else nc.sync
                    eng.dma_start(
                        out=t[r0:128, :],
                        in_=srw_d[s * C + c0 + r0:s * C + c0 + 128, :])
                    row.append(t)
                srw_sb.append(row)

            def bcast(dram_vec, name, eng=None):
                t = consts.tile([128, C], F16, name=name)
                src = bass.AP(tensor=dram_vec.ap().tensor, offset=0,
                              ap=[[0, 128], [1, C]])
                (eng or nc.sync).dma_start(out=t, in_=src)
                return t

            srb_bc = bcast(srb_d, "srb_bc", nc.scalar)
            bv_bc = bcast(bv_d, "bv_bc", nc.scalar)
            bp_bc = bcast(bp_d, "bp_bc", nc.scalar)

            ident = consts.tile([128, 128], F16, name="ident")
            make_identity(nc, ident)
            eps_t = consts.tile([128, 1], F32, name="eps_t")
            nc.vector.memset(eps_t, LN_EPS)

            def load_w_chunks(dram, name):
                ts = []
                for i, (c0, r0) in enumerate(CCHUNKS):
                    t = wpool.tile([128, C], F16, name=f"{name}{i}")
                    nc.sync.dma_start(out=t[r0:128, :],
                                      in_=dram[c0 + r0:c0 + 128, :])
                    ts.append(t)
                return ts

            wq_sb = load_w_chunks(wq_d, "wq")
            wk_sb = load_w_chunks(wk_d, "wk")
            wv_sb = load_w_chunks(wv_d, "wv")
            wp_o = []
            bk_col = []
            for i, (o0, osz) in enumerate(OCHUNKS):
                t = wpool.tile([osz, C], F16, name=f"wp{i}")
                nc.sync.dma_start(out=t, in_=wp_d[o0:o0 + osz, :])
                wp_o.append(t)
                b = wpool.tile([osz, 1], F32, name=f"bk{i}")
                nc.sync.dma_start(out=b, in_=bk_d[o0:o0 + osz].unsqueeze(1))
                bk_col.append(b)

            # v augmented: ones col 0 (-> denominators at psum partition 0),
            # zeros 1:64, v at 64:128
            v_aug = bigp.tile([128, 8, NH, 128], F16, name="v_aug")
            nc.vector.memset(v_aug[:, :, :, 0:64], 0.0)
            nc.vector.memset(v_aug[:, :, :, 0:1], 1.0)

            # ---------------- conv / LN / kT / v prologue ----------------
            lnT = [bigp.tile([128, NP], F16, name=f"lnT{i}") for i in range(3)]
            kT = [bigp.tile([osz, NP], F16, name=f"kT{i}")
                  for i, (_o0, osz) in enumerate(OCHUNKS)]
            ln_tiles = [None] * 8

            def emit_conv(it):
                # prologue psum lives in the (still idle) score regions:
                # bank 0 = conv, bank 1 = v, rotating by kv-chunk parity
                pc = scAB[it % 2][:, 0:C]
                first = True
                for s in range(4):
                    for cc, (_c0, r0) in enumerate(CCHUNKS):
                        nc.tensor.matmul(
                            pc,
                            pat_sb[s][cc][r0:128, it * 128:(it + 1) * 128],
                            srw_sb[s][cc][r0:128, :],
                            start=first, stop=(s == 3 and cc == 2))
                        first = False
                nc.vector.tensor_add(pc, pc, srb_bc)
                stats = small_p.tile([128, 6], F32, name="stats", tag="st")
                nc.vector.bn_stats(stats, pc)
                mv = small_p.tile([128, 2], F32, name="mv", tag="st")
                nc.vector.bn_aggr(mv, stats)
                std = small_p.tile([128, 1], F32, name="std", tag="st")
                nc.scalar.activation(std, mv[:, 1:2],
                                     mybir.ActivationFunctionType.Sqrt, bias=eps_t)
                rstd = small_p.tile([128, 1], F32, name="rstd", tag="st")
                nc.vector.reciprocal(rstd, std)
                ln_h = small_p.tile([128, C], F16, name="ln_h", tag="lnf")
                nc.vector.tensor_scalar(ln_h, pc, mv[:, 0:1], rstd,
                                        op0=mybir.AluOpType.subtract,
                                        op1=mybir.AluOpType.mult)
                ln_tiles[it] = ln_h

            def emit_lnT(it):
                ln_h = ln_tiles[it]
                for ci, (c0, r0) in enumerate(CCHUNKS):
                    pt = ps_av.tile([128, 128], F16, name="pt", tag="a")
                    nc.tensor.transpose(pt, ln_h[:, c0:c0 + 128], ident)
                    nc.vector.tensor_copy(lnT[ci][:, it * 128:(it + 1) * 128], pt)

            def emit_v(it):
                pv = scAB[it % 2][:, 512:512 + C]
                for ci, (_c0, r0) in enumerate(CCHUNKS):
                    nc.tensor.matmul(pv, lnT[ci][r0:128, it * 128:(it + 1) * 128],
                                     wv_sb[ci][r0:128, :],
                                     start=(ci == 0), stop=(ci == 2))
                nc.vector.tensor_add(
                    v_aug[:, it, :, 64:],
                    pv.rearrange("p (h d) -> p h d", h=NH),
                    bv_bc.rearrange("p (h d) -> p h d", h=NH))

            def emit_kT_all():
                for i, (o0, osz) in enumerate(OCHUNKS):
                    for nb in range(2):
                        pk = scAB[(i * 2 + nb) % 2][0:osz, 1024:1536]
                        for ci, (_c0, r0) in enumerate(CCHUNKS):
                            nc.tensor.matmul(
                                pk,
                                wk_sb[ci][r0:128, o0:o0 + osz],
                                lnT[ci][r0:128, nb * 512:(nb + 1) * 512],
                                start=(ci == 0), stop=(ci == 2))
                        nc.vector.tensor_scalar_add(
                            kT[i][:, nb * 512:(nb + 1) * 512], pk, bk_col[i])

            # q^T in permuted token order: qT[:, s*1024 + p] from pat
            qT = [bigp.tile([osz, N], F16, name=f"qT{i}")
                  for i, (_o0, osz) in enumerate(OCHUNKS)]

            def emit_qproj(s, qhb, carve=False):
                for i, (o0, osz) in enumerate(OCHUNKS):
                    if carve:
                        pq = scAB[(s * 2 + qhb + i) % 2][0:osz, 512:1024]
                    else:
                        pq = ps_av.tile([osz, 512], F32, name="pq", tag="a")
                    for ci, (_c0, r0) in enumerate(CCHUNKS):
                        nc.tensor.matmul(
                            pq,
                            wq_sb[ci][r0:128, o0:o0 + osz],
                            pat_sb[s][ci][r0:128, qhb * 512:(qhb + 1) * 512],
                            start=(ci == 0), stop=(ci == 2))
                    nc.vector.tensor_copy(
                        qT[i][:, s * 1024 + qhb * 512:
                              s * 1024 + (qhb + 1) * 512], pq)

            # conv first (only needs pat+srw); lnT/v/kT trail one step;
            # qproj for qb0 (taps 0,1) at the end; taps 2,3 via fillers.
            for it in range(8):
                emit_conv(it)
                if it > 0:
                    emit_lnT(it - 1)
                    emit_v(it - 1)
            emit_lnT(7)
            emit_v(7)
            emit_kT_all()
            for s in (0, 1):
                for qhb in range(2):
                    emit_qproj(s, qhb, carve=True)

            # ============ attention: ACT-saturated region pipeline =========
            # Scores for (qb, h) are computed in 512-col blocks (one MM
            # each) into two rotating 3-bank PSUM regions; each region is
            # exp'd with one [128, 1536] ACTIVATE into its own s_exp tile.
            # PE slack between score groups is filled from a FIFO work
            # queue (AV accumulation of already-exp'd column groups, the
            # output projection, leftover q projection).
            s_exp = {}      # (qb, pi, region) -> se tile
            blk_idx = {}    # (qb, pi, h, qh, kvc) -> block index in unit
            attnT = {}      # qb -> 3 chunk tiles
            parity = [0]
            work_q = []     # FIFO of (pe_cycles, emit_fn)
            PAIRS = [(0, 1), (2, 3), (4,)]

            def se_slice(qb, pi, h, qh, kvc):
                b = blk_idx[(qb, pi, h, qh, kvc)]
                r, j = divmod(b, 3)
                t = s_exp[(qb, pi, r)]
                return t[:, j * 512:(j + 1) * 512]

            def emit_norm2(qb, h, qh, pav):
                ht, hr = h // 2, (h % 2) * 64
                rec = norm_p.tile([1, 512], F32, name="rec", tag="rc")
                nc.vector.reciprocal_approx_fast(rec, pav[0:1, :])
                rec16 = norm_p.tile([1, 512], F16, name="rec16", tag="rch")
                nc.vector.tensor_copy(rec16, rec)
                rb = norm_p.tile([HD, 512], F16, name="rb", tag="rb")
                nc.gpsimd.partition_broadcast(rb, rec16)
                nc.vector.tensor_mul(
                    attnT[qb][ht][hr:hr + HD, qh * 512:(qh + 1) * 512],
                    pav[64:, :], rb)

            def enqueue_av(qb, pi, h, qh):
                box = []
                for kvc in range(8):
                    def mm(kvc=kvc, qb=qb, pi=pi, h=h, qh=qh, box=box):
                        if kvc == 0:
                            box.append(ps_av.tile([128, 512], F32,
                                                  name="pav", tag="a"))
                        nc.tensor.matmul(
                            box[0], v_aug[:, kvc, h, :],
                            se_slice(qb, pi, h, qh, kvc),
                            start=(kvc == 0), stop=(kvc == 7))
                    work_q.append((540, mm))
                def nrm(qb=qb, h=h, qh=qh, box=box):
                    emit_norm2(qb, h, qh, box[0])
                work_q.append((80, nrm))

            def enqueue_qproj(s):
                for qhb in range(2):
                    def qp(s=s, qhb=qhb):
                        emit_qproj(s, qhb)
                    work_q.append((1650, qp))

            def enqueue_proj(qb):
                base, size = QBS[qb]
                for qs in range(size // 128):
                    box = []
                    for ci in range(3):
                        def mm(qb=qb, qs=qs, ci=ci, box=box):
                            if ci == 0:
                                box.append(ps_av.tile([128, 512], F32,
                                                      name="po", tag="a"))
                            nc.tensor.matmul(
                                box[0][:, 0:C],
                                attnT[qb][ci][:, qs * 128:(qs + 1) * 128],
                                wp_o[ci], start=(ci == 0), stop=(ci == 2))
                        work_q.append((360, mm))
                    def ev(qb=qb, qs=qs, base=base, box=box):
                        o_sb = out_p.tile([128, C], F16, name="o_sb", tag="o")
                        nc.vector.tensor_add(o_sb, box[0][:, 0:C], bp_bc)
                        row = base + qs * 128
                        nc.sync.dma_start(out=out_d[row:row + 128, :],
                                          in_=o_sb)
                    work_q.append((80, ev))

            def pop_fillers(budget):
                while work_q and budget > 0:
                    cost, fn = work_q.pop(0)
                    fn()
                    budget -= cost

            def emit_unit(qb, pi):
                base, size = QBS[qb]
                nqh = size // 512
                heads = PAIRS[pi]
                attnT.setdefault(qb, [
                    attn_p.tile([osz, QBB], F16, name=f"aT{qb}_{i}",
                                tag=f"attn{i}")
                    for i, (_o0, osz) in enumerate(OCHUNKS)])
                # qh-major so qh column groups finish early (their AV joins
                # the filler queue within the unit); heads interleaved so
                # consecutive score MMs hit different PE row groups and run
                # concurrently (row tiling, K=64 each).
                blocks = [(hh, qh, kvc)
                          for qh in range(nqh) for kvc in range(8)
                          for hh in heads]
                for bi, (hh, qh, kvc) in enumerate(blocks):
                    blk_idx[(qb, pi, hh, qh, kvc)] = bi
                pos = 0
                for r, rsize in enumerate(_regs(len(blocks))):
                    sc = scAB[parity[0]]
                    parity[0] ^= 1
                    se = sexp_p.tile([128, rsize * 512], F16,
                                     name=f"se{qb}_{pi}_{r}", tag="se")
                    s_exp[(qb, pi, r)] = se
                    for j in range(rsize):
                        hh, qh, kvc = blocks[pos + j]
                        ht, hr = hh // 2, (hh % 2) * 64
                        nc.tensor.matmul(
                            sc[:, j * 512:(j + 1) * 512],
                            kT[ht][hr:hr + HD, kvc * 128:(kvc + 1) * 128],
                            qT[ht][hr:hr + HD, base + qh * 512:
                                   base + (qh + 1) * 512],
                            start=True, stop=True)
                    nc.scalar.activation(
                        se, sc[:, 0:rsize * 512],
                        mybir.ActivationFunctionType.Exp,
                        bias=ebias_t, scale=SCALE)
                    new_pos = pos + rsize
                    for hh in heads:
                        for qh in range(nqh):
                            done = blk_idx[(qb, pi, hh, qh, 7)]
                            if pos <= done < new_pos:
                                enqueue_av(qb, pi, hh, qh)
                    pos = new_pos
                    pop_fillers((rsize * 512 + 352) * 2 - rsize * 259)

            for qb in range(len(QBS)):
                for pi in range(len(PAIRS)):
                    if qb == 0 and pi == 0:
                        enqueue_qproj(2)
                        enqueue_qproj(3)
                    emit_unit(qb, pi)
                enqueue_proj(qb)
            while work_q:
                _cost, fn = work_q.pop(0)
                fn()

            if dbg:
                for i, (_o0, osz) in enumerate(OCHUNKS):
                    nc.sync.dma_start(out=dbg_d[f"dbg_qt{i}"][0:osz, :],
                                      in_=qT[i])
                    nc.sync.dma_start(out=dbg_d[f"dbg_kt{i}"][0:osz, :],
                                      in_=kT[i])
                    nc.sync.dma_start(out=dbg_d[f"dbg_at{i}"][0:osz, :],
                                      in_=attnT[0][i])

    nc.compile()
    return nc


_CACHE = {}


def _get_nc():
    if "nc" not in _CACHE:
        _CACHE["nc"] = build_bass()
    return _CACHE["nc"]


# permuted token order: p = s*1024 + h'*32 + w'  ->  orig (2h'+dh)*64+2w'+dw
def _perm():
    p = np.empty(N, np.int64)
    i = 0
    for s in range(4):
        dh, dw = s // 2, s % 2
        for hp in range(32):
            for wp in range(32):
                p[i] = (2 * hp + dh) * 64 + (2 * wp + dw)
                i += 1
    return p


PERM = _perm()


def make_in_maps(x, Wq, Wkv, sr_w, sr_b, ln_g, ln_b, Wp, bp):
    B = x.shape[0]
    f16 = np.float16
    f32 = np.float32
    ln_g = np.asarray(ln_g, f32)
    ln_b = np.asarray(ln_b, f32)
    wk_f = np.asarray(Wkv[:, :C], f32)
    wv_f = np.asarray(Wkv[:, C:], f32)
    wq = np.ascontiguousarray(Wq, dtype=f16)
    # fold LN gamma/beta into the K/V projections:
    #   (ln*g + b) @ W = ln @ (g[:,None]*W) + b @ W
    wk = np.ascontiguousarray(ln_g[:, None] * wk_f, dtype=f16)
    wv = np.ascontiguousarray(ln_g[:, None] * wv_f, dtype=f16)
    bk = np.ascontiguousarray(ln_b @ wk_f, dtype=f32)
    bv = np.ascontiguousarray(ln_b @ wv_f, dtype=f16)
    srw = np.ascontiguousarray(np.asarray(sr_w, dtype=f16).reshape(4 * C, C))
    wp = np.ascontiguousarray(Wp, dtype=f16)
    srb = np.ascontiguousarray(sr_b, dtype=f16)
    bpv = np.ascontiguousarray(bp, dtype=f16)

    maps = []
    for i in range(B):
        xi = np.asarray(x[i], dtype=f16)          # [N, C]
        patm = np.ascontiguousarray(
            xi[PERM].reshape(4, NP, C).transpose(0, 2, 1).reshape(4 * C, NP))
        maps.append({"pat": patm, "wq": wq, "wk": wk,
                     "wv": wv, "srw": srw, "wp": wp, "srb": srb, "bk": bk,
                     "bv": bv, "bp": bpv})
    return maps


def kernel(x, Wq, Wkv, sr_w, sr_b, ln_g, ln_b, Wp, bp, H=64, W=64):
    x = np.asarray(x, dtype=np.float32)
    B = x.shape[0]
    assert x.shape == (B, N, C), x.shape
    nc = _get_nc()
    in_maps = make_in_maps(x, Wq, Wkv, sr_w, sr_b, ln_g, ln_b, Wp, bp)
    res = run_bass_kernel_spmd(nc, in_maps, core_ids=list(range(8)))
    out = np.empty((B, N, C), np.float32)
    for i in range(B):
        out[i, PERM, :] = res.results[i]["out"].astype(np.float32)
    return out


# revision 45
# speedup vs baseline: 1.2242x; 1.2242x over previous
"""PVT-style spatial-reduction attention on 8 Trainium2 NeuronCores.

Sharding: data-parallel over batch (B=8 -> one batch element per core).
Each core runs the full attention for its batch element; weights are
replicated. No collectives needed.

Token space is PERMUTED on device: the host ships x as conv patches
pat[s*C+c, p] = x^T[c, token(s, p)] (s = 2x2 tap index, p = patch
position), so the conv and the q projection read the same tensor and
no separate x^T upload/transpose is needed. All q-indexed tensors
(qT, scores, attnT, out) live in permuted token order; the host
un-permutes the output rows.

Per-core pipeline (matmuls f16, f32 PSUM accumulation):
  conv2x2s2 + LayerNorm -> ln (kv tokens), lnT via PE transpose
  kT = Wk^T lnT ; v = ln @ Wv (v_aug carries a ones column -> softmax
  denominators fall out of the attn@v matmul)
  scores S^T = kT.T qT in 512-col blocks -> two 3-bank PSUM regions;
  exp([128,1536]) per region on the Scalar engine (the ~200us floor
  this kernel hides everything under)
  attn@v accumulated kvc-wise; 1/denom via DVE reciprocal + gpsimd
  partition broadcast; out = attnT.T @ Wp + bp, f16 DMA out.
"""

import os
import sys
import numpy as np

for _p in ("/opt/trn_rl_repo", "/root/.axon_site/_ro/trn_rl_repo"):
    if os.path.isdir(_p) and _p not in sys.path:
        sys.path.append(_p)

import concourse.bacc as bacc
import concourse.bass as bass
import concourse.mybir as mybir
import concourse.tile as tile
from concourse.bass_utils import run_bass_kernel_spmd
from concourse.masks import make_identity

F16 = mybir.dt.float16
F32 = mybir.dt.float32

N = 4096          # q tokens (H*W = 64*64)
C = 320           # model dim
NH = 5            # heads
HD = 64           # head dim
NP = 1024         # kv tokens ((H/2)*(W/2))
LN_EPS = 1e-3
SCALE = HD ** -0.5
EXP_BIAS = -3.0   # constant shift inside exp; cancels in softmax

# contraction chunks over C=320: three tiles; the last holds c 256:320
# in partitions 64:128 (so tile_position/base checks line up).
CCHUNKS = [(0, 0), (128, 0), (192, 64)]  # (c_start, row0); rows r0:128 used
OCHUNKS = [(0, 128), (128, 128), (256, 64)]

QBB = 2048        # max q block per attention unit
# attention q blocks (base, size): a small final block keeps the un-
# overlapped AV+proj tail short
QBS = [(0, 2048), (2048, 1536), (3584, 512)]


def _regs(nblocks):
    r = [3] * (nblocks // 3)
    if nblocks % 3:
        r.append(nblocks % 3)
    return r


def build_bass(dbg=False):
    nc = bacc.Bacc("TRN2", target_bir_lowering=False, debug=False, num_devices=8)

    pat_d = nc.declare_dram_parameter("pat", [4 * C, NP], F16, isOutput=False)
    wq_d = nc.declare_dram_parameter("wq", [C, C], F16, isOutput=False)
    wk_d = nc.declare_dram_parameter("wk", [C, C], F16, isOutput=False)
    wv_d = nc.declare_dram_parameter("wv", [C, C], F16, isOutput=False)
    srw_d = nc.declare_dram_parameter("srw", [4 * C, C], F16, isOutput=False)
    wp_d = nc.declare_dram_parameter("wp", [C, C], F16, isOutput=False)
    srb_d = nc.declare_dram_parameter("srb", [C], F16, isOutput=False)
    bk_d = nc.declare_dram_parameter("bk", [C], F32, isOutput=False)
    bv_d = nc.declare_dram_parameter("bv", [C], F16, isOutput=False)
    bp_d = nc.declare_dram_parameter("bp", [C], F16, isOutput=False)
    out_d = nc.declare_dram_parameter("out", [N, C], F16, isOutput=True)
    if dbg:
        dbg_d = {
            nm: nc.declare_dram_parameter(nm, shp, F16, isOutput=True)
            for nm, shp in (
                [(f"dbg_qt{i}", [128, N]) for i in range(3)]
                + [(f"dbg_kt{i}", [128, NP]) for i in range(3)]
                + [(f"dbg_at{i}", [128, QBB]) for i in range(3)]
            )
        }

    with tile.TileContext(nc) as tc:
        with (
            tc.tile_pool(name="consts", bufs=1) as consts,
            tc.tile_pool(name="wpool", bufs=1) as wpool,
            tc.tile_pool(name="big", bufs=1) as bigp,
            tc.tile_pool(name="sexp", bufs=24) as sexp_p,
            tc.tile_pool(name="attn", bufs=2) as attn_p,
            tc.tile_pool(name="small", bufs=4) as small_p,
            tc.tile_pool(name="normp", bufs=2) as norm_p,
            tc.tile_pool(name="outp", bufs=3) as out_p,
            # PSUM: two 3-bank score regions + one 2-bank rotating pool
            # for everything else = exactly 8 banks.
            tc.tile_pool(name="ps_av", bufs=2, space="PSUM") as ps_av,
        ):
            # Prologue PSUM: six independent single-bank tiles so the
            # conv/LN/v/kT/qproj pipeline is never serialized by buffer
            # reuse. Freed before the attention score regions (scAB) are
            # allocated in their place.
            scAB = []
            pro = []
            pro_free = []
            for j in range(6):
                t, fr = tc.tile([128, 512], F32, space="PSUM",
                                name=f"pro{j}")
                pro.append(t)
                pro_free.append(fr)

            # PE warmup: junk matmuls before any DMA lands, so the HAM
            # clock gate sees a busy PE and lifts to 2.4 GHz early.
            warm_sb = consts.tile([128, 512], F16, name="warm_sb")
            nc.vector.memset(warm_sb, 0.0)
            for _w in range(12):
                nc.tensor.matmul(pro[5], warm_sb[:, 0:128], warm_sb,
                                 start=True, stop=True)

            # preload the exp activation table (~2.7us) during the DMA wait
            ebias_t = consts.tile([128, 1], F32, name="ebias_t")
            nc.vector.memset(ebias_t, EXP_BIAS)
            warm_act = consts.tile([128, 2], F16, name="warm_act")
            nc.scalar.activation(warm_act, ebias_t.broadcast_to([128, 2]),
                                 mybir.ActivationFunctionType.Exp,
                                 bias=ebias_t, scale=1.0)

            # conv patches / permuted x^T, per (tap s, c-chunk cc), split
            # across both HWDGE queues. This is the only copy of x.
            pat_sb = []   # [s][cc] -> tile [128, NP] (cc2 rows at 64:128)
            for s in range(4):
                row = []
                for cc, (c0, r0) in enumerate(CCHUNKS):
                    t = bigp.tile([128, NP], F16, name=f"pat{s}_{cc}")
                    eng = nc.sync if (s * 3 + cc) % 2 else nc.scalar
                    eng.dma_start(
                        out=t[r0:128, :],
                        in_=pat_d[s * C + c0 + r0:s * C + c0 + 128, :])
                    row.append(t)
                pat_sb.append(row)

            # sr_w per (s, cc) with rows matching pat chunk partitions
            srw_sb = []
            for s in range(4):
                row = []
                for cc, (c0, r0) in enumerate(CCHUNKS):
                    t = wpool.tile([128, C], F16, name=f"srw{s}_{cc}")
                    eng = nc.scalar if (s * 3 + cc) % 2 else nc.sync
                    eng.dma_start(
                        out=t[r0:128, :],
                        in_=srw_d[s * C + c0 + r0:s * C + c0 + 128, :])
                    row.append(t)
                srw_sb.append(row)

            def bcast(dram_vec, name, eng=None):
                t = consts.tile([128, C], F16, name=name)
                src = bass.AP(tensor=dram_vec.ap().tensor, offset=0,
                              ap=[[0, 128], [1, C]])
                (eng or nc.sync).dma_start(out=t, in_=src)
                return t

            srb_bc = bcast(srb_d, "srb_bc", nc.scalar)
            bv_bc = bcast(bv_d, "bv_bc", nc.scalar)
            bp_bc = bcast(bp_d, "bp_bc", nc.scalar)

            ident = consts.tile([128, 128], F16, name="ident")
            make_identity(nc, ident)
            eps_t = consts.tile([128, 1], F32, name="eps_t")
            nc.vector.memset(eps_t, LN_EPS)

            def load_w_chunks(dram, name):
                ts = []
                for i, (c0, r0) in enumerate(CCHUNKS):
                    t = wpool.tile([128, C], F16, name=f"{name}{i}")
                    nc.sync.dma_start(out=t[r0:128, :],
                                      in_=dram[c0 + r0:c0 + 128, :])
                    ts.append(t)
                return ts

            wq_sb = load_w_chunks(wq_d, "wq")
            wk_sb = load_w_chunks(wk_d, "wk")
            wv_sb = load_w_chunks(wv_d, "wv")
            wp_o = []
            bk_col = []
            for i, (o0, osz) in enumerate(OCHUNKS):
                t = wpool.tile([osz, C], F16, name=f"wp{i}")
                nc.sync.dma_start(out=t, in_=wp_d[o0:o0 + osz, :])
                wp_o.append(t)
                b = wpool.tile([osz, 1], F32, name=f"bk{i}")
                nc.sync.dma_start(out=b, in_=bk_d[o0:o0 + osz].unsqueeze(1))
                bk_col.append(b)

            # v augmented: ones col 0 (-> denominators at psum partition 0),
            # zeros 1:64, v at 64:128
            v_aug = bigp.tile([128, 8, NH, 128], F16, name="v_aug")
            nc.vector.memset(v_aug[:, :, :, 0:64], 0.0)
            nc.vector.memset(v_aug[:, :, :, 0:1], 1.0)

            # ---------------- conv / LN / kT / v prologue ----------------
            lnT = [bigp.tile([128, NP], F16, name=f"lnT{i}") for i in range(3)]
            kT = [bigp.tile([osz, NP], F16, name=f"kT{i}")
                  for i, (_o0, osz) in enumerate(OCHUNKS)]
            ln_tiles = [None] * 8

            def emit_conv(it):
                pc = pro[it % 3][:, 0:C]
                first = True
                for s in range(4):
                    for cc, (_c0, r0) in enumerate(CCHUNKS):
                        nc.tensor.matmul(
                            pc,
                            pat_sb[s][cc][r0:128, it * 128:(it + 1) * 128],
                            srw_sb[s][cc][r0:128, :],
                            start=first, stop=(s == 3 and cc == 2))
                        first = False
                nc.vector.tensor_add(pc, pc, srb_bc)
                stats = small_p.tile([128, 6], F32, name="stats", tag="st")
                nc.vector.bn_stats(stats, pc)
                mv = small_p.tile([128, 2], F32, name="mv", tag="st")
                nc.vector.bn_aggr(mv, stats)
                std = small_p.tile([128, 1], F32, name="std", tag="st")
                nc.scalar.activation(std, mv[:, 1:2],
                                     mybir.ActivationFunctionType.Sqrt, bias=eps_t)
                rstd = small_p.tile([128, 1], F32, name="rstd", tag="st")
                nc.vector.reciprocal(rstd, std)
                ln_h = small_p.tile([128, C], F16, name="ln_h", tag="lnf")
                nc.vector.tensor_scalar(ln_h, pc, mv[:, 0:1], rstd,
                                        op0=mybir.AluOpType.subtract,
                                        op1=mybir.AluOpType.mult)
                ln_tiles[it] = ln_h

            def emit_lnT(it):
                ln_h = ln_tiles[it]
                for ci, (c0, r0) in enumerate(CCHUNKS):
                    pt = ps_av.tile([128, 128], F16, name="pt", tag="a")
                    nc.tensor.transpose(pt, ln_h[:, c0:c0 + 128], ident)
                    nc.vector.tensor_copy(lnT[ci][:, it * 128:(it + 1) * 128], pt)

            def emit_v(it):
                pv = pro[3 + it % 2][:, 0:C]
                for ci, (_c0, r0) in enumerate(CCHUNKS):
                    nc.tensor.matmul(pv, lnT[ci][r0:128, it * 128:(it + 1) * 128],
                                     wv_sb[ci][r0:128, :],
                                     start=(ci == 0), stop=(ci == 2))
                nc.vector.tensor_add(
                    v_aug[:, it, :, 64:],
                    pv.rearrange("p (h d) -> p h d", h=NH),
                    bv_bc.rearrange("p (h d) -> p h d", h=NH))

            def emit_kT_all():
                for i, (o0, osz) in enumerate(OCHUNKS):
                    for nb in range(2):
                        pk = pro[(i * 2 + nb) % 6][0:osz, 0:512]
                        for ci, (_c0, r0) in enumerate(CCHUNKS):
                            nc.tensor.matmul(
                                pk,
                                wk_sb[ci][r0:128, o0:o0 + osz],
                                lnT[ci][r0:128, nb * 512:(nb + 1) * 512],
                                start=(ci == 0), stop=(ci == 2))
                        nc.vector.tensor_scalar_add(
                            kT[i][:, nb * 512:(nb + 1) * 512], pk, bk_col[i])

            # q^T in permuted token order: qT[:, s*1024 + p] from pat
            qT = [bigp.tile([osz, N], F16, name=f"qT{i}")
                  for i, (_o0, osz) in enumerate(OCHUNKS)]

            def emit_qproj(s, qhb, carve=False):
                for i, (o0, osz) in enumerate(OCHUNKS):
                    if carve:
                        pq = pro[(s * 6 + qhb * 3 + i) % 6][0:osz, 0:512]
                    else:
                        pq = ps_av.tile([osz, 512], F32, name="pq", tag="a")
                    for ci, (_c0, r0) in enumerate(CCHUNKS):
                        nc.tensor.matmul(
                            pq,
                            wq_sb[ci][r0:128, o0:o0 + osz],
                            pat_sb[s][ci][r0:128, qhb * 512:(qhb + 1) * 512],
                            start=(ci == 0), stop=(ci == 2))
                    nc.vector.tensor_copy(
                        qT[i][:, s * 1024 + qhb * 512:
                              s * 1024 + (qhb + 1) * 512], pq)

            # conv first (only needs pat+srw); lnT/v/kT trail one step;
            # qproj for qb0 (taps 0,1) at the end; taps 2,3 via fillers.
            for it in range(8):
                emit_conv(it)
                if it > 0:
                    emit_lnT(it - 1)
                    emit_v(it - 1)
            emit_lnT(7)
            emit_v(7)
            emit_kT_all()
            for s in (0, 1):
                for qhb in range(2):
                    emit_qproj(s, qhb, carve=True)
            for fr in reversed(pro_free):
                fr()
            scAB_free = []
            for i in range(2):
                t, _fr = tc.tile([128, 1536], F32, space="PSUM",
                                 name=f"scRegion{i}")
                scAB.append(t)
                scAB_free.append(_fr)

            # ============ attention: ACT-saturated region pipeline =========
            # Scores for (qb, h) are computed in 512-col blocks (one MM
            # each) into two rotating 3-bank PSUM regions; each region is
            # exp'd with one [128, 1536] ACTIVATE into its own s_exp tile.
            # PE slack between score groups is filled from a FIFO work
            # queue (AV accumulation of already-exp'd column groups, the
            # output projection, leftover q projection).
            s_exp = {}      # (qb, pi, region) -> se tile
            blk_idx = {}    # (qb, pi, h, qh, kvc) -> block index in unit
            attnT = {}      # qb -> 3 chunk tiles
            parity = [0]
            work_q = []     # FIFO of (pe_cycles, emit_fn)
            PAIRS = [(0, 1), (2, 3), (4,)]

            def se_slice(qb, pi, h, qh, kvc):
                b = blk_idx[(qb, pi, h, qh, kvc)]
                r, j = divmod(b, 3)
                t = s_exp[(qb, pi, r)]
                return t[:, j * 512:(j + 1) * 512]

            def emit_norm2(qb, h, qh, pav):
                ht, hr = h // 2, (h % 2) * 64
                rec = norm_p.tile([1, 512], F32, name="rec", tag="rc")
                nc.vector.reciprocal_approx_fast(rec, pav[0:1, :])
                rec16 = norm_p.tile([1, 512], F16, name="rec16", tag="rch")
                nc.vector.tensor_copy(rec16, rec)
                rb = norm_p.tile([HD, 512], F16, name="rb", tag="rb")
                nc.gpsimd.partition_broadcast(rb, rec16)
                nc.vector.tensor_mul(
                    attnT[qb][ht][hr:hr + HD, qh * 512:(qh + 1) * 512],
                    pav[64:, :], rb)

            def enqueue_av(qb, pi, h, qh):
                box = []
                for kvc in range(8):
                    def mm(kvc=kvc, qb=qb, pi=pi, h=h, qh=qh, box=box):
                        if kvc == 0:
                            box.append(ps_av.tile([128, 512], F32,
                                                  name="pav", tag="a"))
                        nc.tensor.matmul(
                            box[0], v_aug[:, kvc, h, :],
                            se_slice(qb, pi, h, qh, kvc),
                            start=(kvc == 0), stop=(kvc == 7))
                    work_q.append((540, mm))
                def nrm(qb=qb, h=h, qh=qh, box=box):
                    emit_norm2(qb, h, qh, box[0])
                work_q.append((80, nrm))

            def enqueue_qproj(s):
                for qhb in range(2):
                    def qp(s=s, qhb=qhb):
                        emit_qproj(s, qhb)
                    work_q.append((1650, qp))

            def enqueue_proj(qb):
                base, size = QBS[qb]
                for qs in range(size // 128):
                    box = []
                    for ci in range(3):
                        def mm(qb=qb, qs=qs, ci=ci, box=box):
                            if ci == 0:
                                box.append(ps_av.tile([128, 512], F32,
                                                      name="po", tag="a"))
                            nc.tensor.matmul(
                                box[0][:, 0:C],
                                attnT[qb][ci][:, qs * 128:(qs + 1) * 128],
                                wp_o[ci], start=(ci == 0), stop=(ci == 2))
                        work_q.append((360, mm))
                    def ev(qb=qb, qs=qs, base=base, box=box):
                        o_sb = out_p.tile([128, C], F16, name="o_sb", tag="o")
                        nc.vector.tensor_add(o_sb, box[0][:, 0:C], bp_bc)
                        row = base + qs * 128
                        nc.sync.dma_start(out=out_d[row:row + 128, :],
                                          in_=o_sb)
                    work_q.append((80, ev))

            def pop_fillers(budget):
                while work_q and budget > 0:
                    cost, fn = work_q.pop(0)
                    fn()
                    budget -= cost

            def emit_unit(qb, pi):
                base, size = QBS[qb]
                nqh = size // 512
                heads = PAIRS[pi]
                attnT.setdefault(qb, [
                    attn_p.tile([osz, QBB], F16, name=f"aT{qb}_{i}",
                                tag=f"attn{i}")
                    for i, (_o0, osz) in enumerate(OCHUNKS)])
                # qh-major so qh column groups finish early (their AV joins
                # the filler queue within the unit); heads interleaved so
                # consecutive score MMs hit different PE row groups and run
                # concurrently (row tiling, K=64 each).
                blocks = [(hh, qh, kvc)
                          for qh in range(nqh) for kvc in range(8)
                          for hh in heads]
                for bi, (hh, qh, kvc) in enumerate(blocks):
                    blk_idx[(qb, pi, hh, qh, kvc)] = bi
                pos = 0
                for r, rsize in enumerate(_regs(len(blocks))):
                    sc = scAB[parity[0]]
                    parity[0] ^= 1
                    se = sexp_p.tile([128, rsize * 512], F16,
                                     name=f"se{qb}_{pi}_{r}", tag="se")
                    s_exp[(qb, pi, r)] = se
                    for j in range(rsize):
                        hh, qh, kvc = blocks[pos + j]
                        ht, hr = hh // 2, (hh % 2) * 64
                        nc.tensor.matmul(
                            sc[:, j * 512:(j + 1) * 512],
                            kT[ht][hr:hr + HD, kvc * 128:(kvc + 1) * 128],
                            qT[ht][hr:hr + HD, base + qh * 512:
                                   base + (qh + 1) * 512],
                            start=True, stop=True)
                    nc.scalar.activation(
                        se, sc[:, 0:rsize * 512],
                        mybir.ActivationFunctionType.Exp,
                        bias=ebias_t, scale=SCALE)
                    new_pos = pos + rsize
                    for hh in heads:
                        for qh in range(nqh):
                            done = blk_idx[(qb, pi, hh, qh, 7)]
                            if pos <= done < new_pos:
                                enqueue_av(qb, pi, hh, qh)
                    pos = new_pos
                    pop_fillers((rsize * 512 + 352) * 2 - rsize * 259)

            for qb in range(len(QBS)):
                for pi in range(len(PAIRS)):
                    if qb == 0 and pi == 0:
                        enqueue_qproj(2)
                        enqueue_qproj(3)
                    emit_unit(qb, pi)
                enqueue_proj(qb)
            while work_q:
                _cost, fn = work_q.pop(0)
                fn()
            for fr in reversed(scAB_free):
                fr()

            if dbg:
                for i, (_o0, osz) in enumerate(OCHUNKS):
                    nc.sync.dma_start(out=dbg_d[f"dbg_qt{i}"][0:osz, :],
                                      in_=qT[i])
                    nc.sync.dma_start(out=dbg_d[f"dbg_kt{i}"][0:osz, :],
                                      in_=kT[i])
                    nc.sync.dma_start(out=dbg_d[f"dbg_at{i}"][0:osz, :],
                                      in_=attnT[0][i])

    nc.compile()
    return nc


_CACHE = {}


def _get_nc():
    if "nc" not in _CACHE:
        _CACHE["nc"] = build_bass()
    return _CACHE["nc"]


# permuted token order: p = s*1024 + h'*32 + w'  ->  orig (2h'+dh)*64+2w'+dw
def _perm():
    p = np.empty(N, np.int64)
    i = 0
    for s in range(4):
        dh, dw = s // 2, s % 2
        for hp in range(32):
            for wp in range(32):
                p[i] = (2 * hp + dh) * 64 + (2 * wp + dw)
                i += 1
    return p


PERM = _perm()


def make_in_maps(x, Wq, Wkv, sr_w, sr_b, ln_g, ln_b, Wp, bp):
    B = x.shape[0]
    f16 = np.float16
    f32 = np.float32
    ln_g = np.asarray(ln_g, f32)
    ln_b = np.asarray(ln_b, f32)
    wk_f = np.asarray(Wkv[:, :C], f32)
    wv_f = np.asarray(Wkv[:, C:], f32)
    wq = np.ascontiguousarray(Wq, dtype=f16)
    # fold LN gamma/beta into the K/V projections:
    #   (ln*g + b) @ W = ln @ (g[:,None]*W) + b @ W
    wk = np.ascontiguousarray(ln_g[:, None] * wk_f, dtype=f16)
    wv = np.ascontiguousarray(ln_g[:, None] * wv_f, dtype=f16)
    bk = np.ascontiguousarray(ln_b @ wk_f, dtype=f32)
    bv = np.ascontiguousarray(ln_b @ wv_f, dtype=f16)
    srw = np.ascontiguousarray(np.asarray(sr_w, dtype=f16).reshape(4 * C, C))
    wp = np.ascontiguousarray(Wp, dtype=f16)
    srb = np.ascontiguousarray(sr_b, dtype=f16)
    bpv = np.ascontiguousarray(bp, dtype=f16)

    maps = []
    for i in range(B):
        xi = np.asarray(x[i], dtype=f16)          # [N, C]
        patm = np.ascontiguousarray(
            xi[PERM].reshape(4, NP, C).transpose(0, 2, 1).reshape(4 * C, NP))
        maps.append({"pat": patm, "wq": wq, "wk": wk,
                     "wv": wv, "srw": srw, "wp": wp, "srb": srb, "bk": bk,
                     "bv": bv, "bp": bpv})
    return maps


def kernel(x, Wq, Wkv, sr_w, sr_b, ln_g, ln_b, Wp, bp, H=64, W=64):
    x = np.asarray(x, dtype=np.float32)
    B = x.shape[0]
    assert x.shape == (B, N, C), x.shape
    nc = _get_nc()
    in_maps = make_in_maps(x, Wq, Wkv, sr_w, sr_b, ln_g, ln_b, Wp, bp)
    res = run_bass_kernel_spmd(nc, in_maps, core_ids=list(range(8)))
    out = np.empty((B, N, C), np.float32)
    for i in range(B):
        out[i, PERM, :] = res.results[i]["out"].astype(np.float32)
    return out
